# revision 1
# baseline (speedup 1.0000x reference)
import jax
import jax.numpy as jnp
import numpy as np
from functools import partial

# nn_GatedMultimodalFusion: B=16384, D_IMG=2048, D_TAB=128, D=512
# Pure data parallel over 8 NeuronCores: batch sharded, weights replicated.
B, D_IMG, D_TAB, D = 16384, 2048, 128, 512
N_CORES = 8
EPS = 1e-5

WEIGHT_KEYS = (
    "Wi", "bi", "ln_i_g", "ln_i_b",
    "Wt", "bt", "ln_t_g", "ln_t_b",
    "Wgi", "bgi", "Wgt", "bgt",
    "Win", "bin_proj", "Wout", "bout",
    "Wf1", "bf1", "lnf1_g", "lnf1_b",
    "Wf2", "bf2", "lnf2_g", "lnf2_b",
)


def _layer_norm(x, g, b):
    mu = jnp.mean(x, axis=-1, keepdims=True)
    var = jnp.var(x, axis=-1, keepdims=True)
    return (x - mu) * jax.lax.rsqrt(var + EPS) * g + b


def _shard_body(img, tab, w):
    img_lin = img @ w["Wi"].T + w["bi"]
    tab_lin = tab @ w["Wt"].T + w["bt"]
    img_proj = jax.nn.gelu(_layer_norm(img_lin, w["ln_i_g"], w["ln_i_b"]),
                           approximate=False)
    tab_proj = jax.nn.gelu(_layer_norm(tab_lin, w["ln_t_g"], w["ln_t_b"]),
                           approximate=False)

    img_gated = img_proj * jax.nn.sigmoid(img_proj @ w["Wgi"].T + w["bgi"])
    tab_gated = tab_proj * jax.nn.sigmoid(tab_proj @ w["Wgt"].T + w["bgt"])

    # seq_len==1 MHA: softmax over one key is 1, so output == out_proj(v_proj(kv))
    Wv = w["Win"][2 * D:3 * D]
    bv = w["bin_proj"][2 * D:3 * D]
    img_att = (tab_gated @ Wv.T + bv) @ w["Wout"].T + w["bout"]
    tab_att = (img_gated @ Wv.T + bv) @ w["Wout"].T + w["bout"]

    combined = jnp.concatenate([img_att, tab_att], axis=1)
    h = jax.nn.gelu(
        _layer_norm(combined @ w["Wf1"].T + w["bf1"], w["lnf1_g"], w["lnf1_b"]),
        approximate=False)
    fused = _layer_norm(h @ w["Wf2"].T + w["bf2"], w["lnf2_g"], w["lnf2_b"])
    return fused + img_gated + tab_gated


_pmapped = None


def _get_pmapped():
    global _pmapped
    if _pmapped is None:
        devs = jax.devices()[:N_CORES]
        _pmapped = jax.pmap(_shard_body, in_axes=(0, 0, None), devices=devs)
    return _pmapped


def kernel(**inputs) -> np.ndarray:
    img = np.asarray(inputs["image_features"], dtype=np.float32)
    tab = np.asarray(inputs["tabular_features"], dtype=np.float32)
    w = {k: jnp.asarray(np.asarray(inputs[k], dtype=np.float32))
         for k in WEIGHT_KEYS}

    img_sh = img.reshape(N_CORES, B // N_CORES, D_IMG)
    tab_sh = tab.reshape(N_CORES, B // N_CORES, D_TAB)

    out = _get_pmapped()(img_sh, tab_sh, w)
    out = np.asarray(out).reshape(B, D)
    return out

